# revision 1
# baseline (speedup 1.0000x reference)
"""Video attention (L=2048, D=1024, 16 heads) on 8 Trainium2 NeuronCores.

Sharding: tensor-parallel over heads. Each core owns 2 heads (= 128 of the
1024 channels): Wq/Wk/Wv are split column-wise by head, Wo row-wise; each
core emits a full-shape partial output and the host sums the 8 partials
(the "all-reduce after to_out" done at gather time).

Per-core kernel layout choices:
  - activations kept transposed [channel, token] so every matmul contracts
    along partitions at full PE rate (float32r, 1 cycle/row at N>=512)
  - scores computed transposed [key j, query i]; softmax exp is layout
    agnostic; the denominator comes for free from a ones-augmented 65th
    column of V in the AV matmul (row 64 of the accumulator = sumexp)
  - RoPE rotate_half implemented as a constant 128x128 sign-permutation
    matmul, with cos/sin multiplies on the vector engine
"""

import os
import sys

import numpy as np

sys.path.insert(0, "/opt/trn_rl_repo")

import concourse.bacc as bacc
import concourse.bass as bass
import concourse.mybir as mybir
import concourse.tile as tile
from concourse.bass_utils import run_bass_kernel_spmd
from concourse.masks import make_identity

F32 = mybir.dt.float32
F32R = mybir.dt.float32r

T, H, W, B, D = 2, 32, 32, 1, 1024
NH = 16
HD = D // NH          # 64
L = T * H * W         # 2048
NCORES = 8
C = D // NCORES       # 128 channels (2 heads) per core
NT = L // 512         # 4 token tiles of 512
KT = D // 128         # 8 contraction tiles for projections
JT = L // 128         # 16 key tiles of 128


def build_program(loop_iters=None):
    nc = bacc.Bacc("TRN2", target_bir_lowering=False, debug=False)

    xT = nc.dram_tensor("xT", [D, L], F32R, kind="ExternalInput")
    wq = nc.dram_tensor("wq", [D, C], F32R, kind="ExternalInput")
    wk = nc.dram_tensor("wk", [D, C], F32R, kind="ExternalInput")
    wv = nc.dram_tensor("wv", [D, C], F32R, kind="ExternalInput")
    wo = nc.dram_tensor("wo", [C, D], F32R, kind="ExternalInput")
    cosT = nc.dram_tensor("cosT", [C, L], F32R, kind="ExternalInput")
    sinT = nc.dram_tensor("sinT", [C, L], F32R, kind="ExternalInput")
    rmat = nc.dram_tensor("rmat", [128, 128], F32R, kind="ExternalInput")
    vones = nc.dram_tensor("vones", [128, JT * 130], F32R, kind="ExternalInput")
    out = nc.dram_tensor("out", [L, D], F32, kind="ExternalOutput")

    with tile.TileContext(nc) as tc:
        with (
            tc.tile_pool(name="res", bufs=1) as res,
            tc.tile_pool(name="sbw", bufs=3) as sbw,
            tc.tile_pool(name="exp", bufs=4) as expp,
            tc.tile_pool(name="ypool", bufs=2) as ypool,
            tc.tile_pool(name="scl", bufs=2) as sclp,
            tc.tile_pool(name="psum", bufs=2, space="PSUM") as pps,
        ):
            def emit_body():
                # ---- resident SBUF tensors ----
                # DMA order matters: q-weights first, then x k-tiles in the
                # order QKV consumes them, so PE starts ~7us in instead of
                # waiting for the full ~11MB input load.
                wqs = res.tile([128, KT * C], F32R, tag="wq")
                wks = res.tile([128, KT * C], F32R, tag="wk")
                wvs = res.tile([128, KT * C], F32R, tag="wv")
                for wsb, wdr in ((wqs, wq), (wks, wk), (wvs, wv)):
                    nc.sync.dma_start(
                        wsb[:].rearrange("p (k c) -> p k c", k=KT),
                        wdr[:].rearrange("(k p) c -> p k c", p=128),
                    )
                # x: one [128, KT*L] tile; block k holds d-rows 128k..128k+128.
                # Streamed in token-tile chunks so projection n can start as
                # soon as chunk n has landed (a projection contracts all of D).
                xall = res.tile([128, KT * L], F32R, tag="xall")
                for n in range(NT):
                    nsl = slice(512 * n, 512 * (n + 1))
                    nc.sync.dma_start(
                        xall[:].rearrange("p (k l) -> p k l", k=KT)[:, :, nsl],
                        xT[:, nsl].rearrange("(k p) l -> p k l", p=128),
                    )
                xs = [xall[:, L * k:L * (k + 1)] for k in range(KT)]
                rms = res.tile([128, 128], F32R, tag="rm")
                nc.sync.dma_start(rms[:], rmat[:])
                coss = res.tile([128, L], F32R, tag="cos")
                sins = res.tile([128, L], F32R, tag="sin")
                nc.sync.dma_start(coss[:], cosT[:])
                nc.sync.dma_start(sins[:], sinT[:])
                wos = res.tile([128, D], F32R, tag="wo")
                nc.sync.dma_start(wos[:], wo[:])
                ident = res.tile([128, 128], F32, tag="id")
                make_identity(nc, ident[:])

                qns = res.tile([128, L], F32R, tag="qn")
                kns = res.tile([128, L], F32R, tag="kn")
                # v in natural [token, c] layout, 65-wide per head (65th col = 1.0)
                vaug = res.tile([128, JT * 130], F32R, tag="vaug")
                nc.sync.dma_start(vaug[:], vones[:])

                # ---- phase A: projections + rope + v transpose ----
                # q/k/v interleaved per token tile so attention (phase B) can
                # begin as soon as token tile 0 is projected
                for n in range(NT):
                    nsl = slice(512 * n, 512 * (n + 1))
                    for wsb, dest in ((wqs, qns), (wks, kns)):
                        ps = pps.tile([128, 512], F32, tag="proj", name=f"p{n}_{dest.name}")
                        for kk in range(KT):
                            nc.tensor.matmul(
                                ps[:],
                                wsb[:, C * kk:C * (kk + 1)],
                                xs[kk][:, nsl],
                                start=(kk == 0),
                                stop=(kk == KT - 1),
                            )
                        craw = sbw.tile([128, 512], F32R, tag="craw")
                        nc.scalar.copy(craw[:], ps[:])
                        rot = pps.tile([128, 512], F32, tag="rot")
                        nc.tensor.matmul(rot[:], rms[:], craw[:], start=True, stop=True)
                        qc = sbw.tile([128, 512], F32R, tag="qc")
                        nc.vector.tensor_mul(qc[:], craw[:], coss[:, nsl])
                        t2 = sbw.tile([128, 512], F32R, tag="t2")
                        nc.vector.tensor_mul(t2[:], rot[:], sins[:, nsl])
                        nc.vector.tensor_add(dest[:, nsl], qc[:], t2[:])

                    ps = pps.tile([128, 512], F32, tag="proj", name=f"pv{n}")
                    for kk in range(KT):
                        nc.tensor.matmul(
                            ps[:],
                            wvs[:, C * kk:C * (kk + 1)],
                            xs[kk][:, nsl],
                            start=(kk == 0),
                            stop=(kk == KT - 1),
                        )
                    vraw = sbw.tile([128, 512], F32, tag="craw")
                    nc.scalar.copy(vraw[:], ps[:])
                    for jj in range(4):
                        j = 4 * n + jj
                        tr = pps.tile([128, 128], F32, tag="u")
                        nc.tensor.transpose(tr[:], vraw[:, 128 * jj:128 * (jj + 1)], ident[:])
                        nc.vector.tensor_copy(vaug[:, 130 * j:130 * j + 64], tr[:, 0:64])
                        nc.vector.tensor_copy(vaug[:, 130 * j + 65:130 * j + 129], tr[:, 64:128])

                # ---- phase B: attention ----
                for i in range(NT):
                    isl = slice(512 * i, 512 * (i + 1))
                    u = [
                        pps.tile([65, 512], F32, tag="u", name=f"u{i}_{h}")
                        for h in range(2)
                    ]
                    for j in range(JT):
                        # both heads' scores packed into one [128,1024] PSUM tile
                        # so a single exp covers them
                        sps = pps.tile([128, 1024], F32, tag="proj", name=f"s{i}_{j}")
                        for h in range(2):
                            hp = slice(64 * h, 64 * (h + 1))
                            nc.tensor.matmul(
                                sps[:, 512 * h:512 * (h + 1)],
                                kns[hp, 128 * j:128 * (j + 1)],
                                qns[hp, isl],
                                start=True,
                                stop=True,
                            )
                        e = expp.tile([128, 1024], F32R, tag="e")
                        nc.scalar.activation(e[:], sps[:], mybir.ActivationFunctionType.Exp)
                        for h in range(2):
                            nc.tensor.matmul(
                                u[h][:],
                                vaug[:, 130 * j + 65 * h:130 * j + 65 * (h + 1)],
                                e[:, 512 * h:512 * (h + 1)],
                                start=(j == 0),
                                stop=(j == JT - 1),
                            )
                    y = ypool.tile([128, 512], F32R, tag="y")
                    for h in range(2):
                        rec = sclp.tile([1, 512], F32, tag="rec")
                        nc.vector.reciprocal(rec[:], u[h][64:65, :])
                        scl = sclp.tile([64, 512], F32, tag="scl")
                        nc.gpsimd.partition_broadcast(scl[:], rec[:])
                        nc.vector.tensor_mul(y[64 * h:64 * (h + 1), :], u[h][0:64, :], scl[:])
                    stage = ypool.tile([128, 4096], F32, tag="stage", name=f"st{i}")
                    for m in range(4):
                        for n2 in range(2):
                            ops_ = pps.tile([128, 512], F32, tag="rot")
                            nc.tensor.matmul(
                                ops_[:],
                                y[:, 128 * m:128 * (m + 1)],
                                wos[:, 512 * n2:512 * (n2 + 1)],
                                start=True,
                                stop=True,
                            )
                            nc.vector.tensor_copy(
                                stage[:, 1024 * m + 512 * n2:1024 * m + 512 * (n2 + 1)],
                                ops_[:],
                            )
                    nc.sync.dma_start(
                        out[512 * i:512 * (i + 1), :].rearrange(
                            "(m p) d -> p m d", p=128),
                        stage[:].rearrange("p (m d) -> p m d", m=4),
                    )

            if loop_iters is None:
                emit_body()
            else:
                with tc.For_i(0, loop_iters, 1):
                    emit_body()

    nc.compile()
    return nc


_NC = None


def _get_nc():
    global _NC
    if _NC is None:
        _NC = build_program()
    return _NC


def make_in_maps(x, rope_emb_L_1_1_D, Wq, Wk, Wv, Wo):
    """Host-side prep: shard weights by head, transpose x, build rope tables."""
    x = np.asarray(x, dtype=np.float32)
    rope = np.asarray(rope_emb_L_1_1_D, dtype=np.float32).reshape(L, HD)
    Wq = np.asarray(Wq, dtype=np.float32)
    Wk = np.asarray(Wk, dtype=np.float32)
    Wv = np.asarray(Wv, dtype=np.float32)
    Wo = np.asarray(Wo, dtype=np.float32)

    xs_flat = x.reshape(L, D)  # B == 1
    xT = np.ascontiguousarray(xs_flat.T)

    cos = np.cos(rope).T  # [HD, L]
    sin = np.sin(rope).T
    cosT = np.ascontiguousarray(np.concatenate([cos, cos], axis=0))  # [128, L]
    sinT = np.ascontiguousarray(np.concatenate([sin, sin], axis=0))

    # rot(q)[d'] = sum_k rmat[k, d'] q[k]; per 64-block: first 32 rows get
    # -q[d+32], last 32 get +q[d-32]  (signs folded in so sinT is plain sin)
    rmat = np.zeros((128, 128), dtype=np.float32)
    for b in (0, 64):
        for m in range(32):
            rmat[b + m + 32, b + m] = -1.0
        for m in range(32, 64):
            rmat[b + m - 32, b + m] = 1.0

    scale = HD ** -0.5
    in_maps = []
    for c in range(NCORES):
        rows = slice(C * c, C * (c + 1))
        in_maps.append({
            "xT": xT,
            "wq": np.ascontiguousarray((scale * Wq[rows, :]).T),
            "wk": np.ascontiguousarray(Wk[rows, :].T),
            "wv": np.ascontiguousarray(Wv[rows, :].T),
            "wo": np.ascontiguousarray(Wo[:, rows].T),
            "cosT": cosT,
            "sinT": sinT,
            "rmat": rmat,
            "vones": np.ones((128, JT * 130), dtype=np.float32),
        })
    return in_maps


class _Runner:
    """Persistent jitted SPMD executable (mirrors bass2jax.run_bass_via_pjrt
    but caches the compiled callable across invocations)."""

    def __init__(self, nc):
        import jax
        from jax.sharding import Mesh, PartitionSpec
        from jax.experimental.shard_map import shard_map
        from concourse import bass2jax

        bass2jax.install_neuronx_cc_hook()
        self.jax = jax
        self.nc = nc
        part_name = nc.partition_id_tensor.name if nc.partition_id_tensor else None
        in_names, out_names, out_avals, zero_shapes = [], [], [], []
        for alloc in nc.m.functions[0].allocations:
            if not isinstance(alloc, mybir.MemoryLocationSet):
                continue
            name = alloc.memorylocations[0].name
            if alloc.kind == "ExternalInput":
                if name != part_name:
                    in_names.append(name)
            elif alloc.kind == "ExternalOutput":
                out_names.append(name)
                shape = tuple(alloc.tensor_shape)
                dtype = mybir.dt.np(alloc.dtype)
                out_avals.append(jax.core.ShapedArray(shape, dtype))
                zero_shapes.append((shape, dtype))
        self.in_names = list(in_names)
        self.out_names = list(out_names)
        self.out_avals = out_avals
        self.zero_shapes = zero_shapes
        n_params = len(in_names)
        n_outs = len(out_names)
        all_in_names = in_names + out_names
        if part_name is not None:
            all_in_names = all_in_names + [part_name]

        def _body(*args):
            operands = list(args)
            if part_name is not None:
                operands.append(bass2jax.partition_id_tensor())
            outs = bass2jax._bass_exec_p.bind(
                *operands,
                out_avals=tuple(out_avals),
                in_names=tuple(all_in_names),
                out_names=tuple(out_names),
                lowering_input_output_aliases=(),
                sim_require_finite=True,
                sim_require_nnan=True,
                nc=nc,
            )
            return tuple(outs)

        devices = jax.devices()[:NCORES]
        self.mesh = Mesh(np.asarray(devices), ("core",))
        self.pspec = PartitionSpec("core")
        in_specs = (self.pspec,) * (n_params + n_outs)
        out_specs = (self.pspec,) * n_outs
        self.sharded = jax.jit(
            shard_map(_body, mesh=self.mesh, in_specs=in_specs,
                      out_specs=out_specs, check_rep=False),
            donate_argnums=tuple(range(n_params, n_params + n_outs)),
            keep_unused=True,
        )

    def concat_inputs(self, in_maps):
        return [
            np.concatenate([np.asarray(m[name]) for m in in_maps], axis=0)
            for name in self.in_names
        ]

    def device_inputs(self, in_maps):
        from jax.sharding import NamedSharding
        sh = NamedSharding(self.mesh, self.pspec)
        return [self.jax.device_put(a, sh) for a in self.concat_inputs(in_maps)]

    def fresh_zeros(self):
        from jax.sharding import NamedSharding
        sh = NamedSharding(self.mesh, self.pspec)
        return [
            self.jax.device_put(
                np.zeros((NCORES * s[0], *s[1:]), dt), sh)
            for s, dt in self.zero_shapes
        ]

    def __call__(self, dev_in, zeros):
        outs = self.sharded(*dev_in, *zeros)
        self.jax.block_until_ready(outs)
        return outs

    def run_np(self, in_maps):
        outs = self(self.device_inputs(in_maps), self.fresh_zeros())
        per_core = []
        for c in range(NCORES):
            d = {}
            for idx, name in enumerate(self.out_names):
                shape = self.out_avals[idx].shape
                d[name] = np.asarray(outs[idx]).reshape(NCORES, *shape)[c]
            per_core.append(d)
        return per_core


_RUNNER = None


def _get_runner():
    global _RUNNER
    if _RUNNER is None:
        _RUNNER = _Runner(_get_nc())
    return _RUNNER


def run(inputs):
    runner = _get_runner()
    in_maps = make_in_maps(**inputs)
    results = runner.run_np(in_maps)
    partial = np.zeros((L, D), dtype=np.float64)
    for r in results:
        partial += r["out"].astype(np.float64)
    full = partial.astype(np.float32).reshape(T, H, W, B, D)
    return full


def kernel(**inputs):
    return run(inputs)

